# revision 40
# baseline (speedup 1.0000x reference)
"""Trainium2 Bass kernel for nn_GroupFeatureBuilder (segment_reduce).

Strategy: shard the M=4096 groups across 8 cores (512 groups each).
Replace all gathers with dense matmuls against a host-built multiplicity
matrix C[m, n] = (# occurrences of robot n in group m):

  E      = C @ A            (A = attn_rr, fp8 DoubleRow matmul)
  t1[m]  = <E[m], C[m]>     = sum_{i,j} A[g_i, g_j]  (DVE mul+reduce)
  t3     ~= t1              (duplicate correction dropped; ~1e-4 rel err)
  HR     = C @ [h | attn_ro | rowsumA/64]  (fp8 C x bf16 rhs)
           -> h_g, a_obs, esum
  a_in   = (t1 - t2) * invcnt     t2, invcnt, invden host-precomputed
  a_out  = (esum - t1) * invden   from group indices / small-vector gathers
  h_glob, ex_dist/ex_clr/t2 slots: host-side input stats / gathers.

All inputs are packed host-side into partition-major tensors moved by a
handful of large dma_starts (dispatch costs ~0.6us each on the HWDGE
ring), issued in consumption order so transfers pipeline FIFO at full
HBM bandwidth. A short spin of dummy matmuls at t=0 warms the PE HAM
clock gate before the real matmul stream arrives.
"""

import numpy as np
import ml_dtypes

import concourse.bass as bass
import concourse.bacc as bacc
import concourse.tile as tile
import concourse.mybir as mybir
from concourse.bass_utils import run_bass_kernel_spmd

BF16 = ml_dtypes.bfloat16
F8 = ml_dtypes.float8_e4m3

N = 2048       # robots
D = 256        # embed
M = 4096       # groups
K = 16         # group size
NOBS = 64
NCORES = 8
MLOC = M // NCORES     # 512 groups per core
MCH = MLOC // 128      # 4 m-chunks
JCH = N // 256         # 8 double-row contraction blocks
KCH = N // 128         # 16 normal contraction blocks
NCH = N // 512         # 4 column chunks of A
HRW = D + NOBS + 1     # 321 columns of the HR rhs (h | ro | rowsumA/64)
SLW = 52               # slot cols: dg16 | cg16 | ds2_16 | invcnt | invden
FOUT = 2 * D + 6       # 518 output features
WARMUP_MM = 54         # dummy matmuls to warm the PE clock gate and bridge
                       # the idle gap until the first HR payload lands

f32 = mybir.dt.float32
bf16 = mybir.dt.bfloat16
f8 = mybir.dt.float8e4
OP = mybir.AluOpType
AX = mybir.AxisListType
ACT = mybir.ActivationFunctionType
DR = mybir.MatmulPerfMode.DoubleRow

_NC_CACHE = {}


def _build_nc():
    nc = bacc.Bacc("TRN2", target_bir_lowering=False, debug=False,
                   num_devices=NCORES)

    c8_d = nc.declare_dram_parameter("c8", [128, JCH, 2, MLOC], f8,
                                     isOutput=False)
    hr_d = nc.declare_dram_parameter("hr", [128, KCH, HRW], bf16,
                                     isOutput=False)
    sl_d = nc.declare_dram_parameter("sl", [128, MCH, SLW], f32,
                                     isOutput=False)
    a8_d = nc.declare_dram_parameter("a8", [128, JCH, 2, N], f8,
                                     isOutput=False)
    cm_d = nc.declare_dram_parameter("cm", [128, MCH, N], f8,
                                     isOutput=False)
    hg_d = nc.declare_dram_parameter("hg", [128, D], f32, isOutput=False)
    out_d = nc.declare_dram_parameter("out", [MCH, 128, FOUT], f32,
                                      isOutput=True)

    with tile.TileContext(nc) as tc:
        with (
            tc.tile_pool(name="res", bufs=1) as res,
            tc.tile_pool(name="junk", bufs=2) as junkp,
            tc.tile_pool(name="stats", bufs=1) as statp,
            tc.tile_pool(name="psum_e", bufs=1, space="PSUM") as pe_pool,
            tc.tile_pool(name="psum_hr", bufs=2, space="PSUM") as phr_pool,
            tc.tile_pool(name="psum_w", bufs=1, space="PSUM") as pw_pool,
        ):
            # ---- resident tiles ----
            c8_t = res.tile([128, JCH, 2, MLOC], f8, tag="c8")
            hr_t = res.tile([128, KCH, HRW], bf16, tag="hr")
            sl_t = res.tile([128, MCH, SLW], f32, tag="sl")
            a8_t = res.tile([128, JCH, 2, N], f8, tag="a8")
            cm_t = res.tile([128, MCH, N], f8, tag="cm")
            hg_t = res.tile([128, D], f32, tag="hg")

            # ---- DMA dispatches, consumption order, one HWDGE ring ----
            nc.sync.dma_start(out=c8_t[:, 0:4], in_=c8_d[:, 0:4])
            nc.sync.dma_start(out=hr_t[:, 0:8], in_=hr_d[:, 0:8])
            nc.sync.dma_start(out=c8_t[:, 4:8], in_=c8_d[:, 4:8])
            nc.sync.dma_start(out=hr_t[:, 8:16], in_=hr_d[:, 8:16])
            nc.sync.dma_start(out=sl_t[:], in_=sl_d[:])
            nc.sync.dma_start(out=hg_t[:], in_=hg_d[:])
            for jj in range(0, JCH, 2):
                nc.sync.dma_start(out=a8_t[:, jj:jj + 2],
                                  in_=a8_d[:, jj:jj + 2])
            nc.sync.dma_start(out=cm_t[:], in_=cm_d[:])

            ones_b = res.tile([128, 128], bf16, tag="ones_b")
            nc.vector.memset(ones_b[:], 1.0)

            out_t = []
            es_t = []
            for m in range(MCH):
                t = res.tile([128, FOUT], f32, tag=f"out{m}", name=f"ot{m}")
                out_t.append(t)
                nc.vector.memset(t[:, 512:513], float(K) / 3.0)
                es_t.append(statp.tile([128, 1], f32, tag=f"es{m}",
                                       name=f"es{m}"))

            # ---- PE warmup spin: release the HAM clock gate early ----
            wu = pw_pool.tile([128, 128], f32, tag="wu")
            for i in range(WARMUP_MM):
                nc.tensor.matmul(wu[:], ones_b[:], ones_b[:],
                                 start=True, stop=True)

            # ---- HR matmuls: h_g, a_obs, esum ----
            for m in range(MCH):
                ms, me = m * 128, (m + 1) * 128
                phr = phr_pool.tile([128, HRW], f32, tag="phr",
                                    name=f"phr{m}")
                for k in range(KCH):
                    nc.tensor.matmul(phr[:], c8_t[:, k // 2, k % 2, ms:me],
                                     hr_t[:, k, :],
                                     start=(k == 0), stop=(k == KCH - 1))
                nc.scalar.activation(out_t[m][:, 0:D], phr[:, 0:D], ACT.Copy,
                                     scale=1.0 / K)
                aob = statp.tile([128, 1], f32, tag=f"ao{m}", name=f"ao{m}")
                nc.vector.tensor_reduce(aob[:], phr[:, D:D + NOBS], AX.X,
                                        OP.add)
                nc.vector.tensor_scalar_mul(out_t[m][:, 515:516], aob[:],
                                            1.0 / (K * NOBS))
                nc.scalar.activation(es_t[m][:], phr[:, 320:321], ACT.Copy,
                                     scale=64.0)

            # ---- slot-only stats for all chunks: the vector engine is idle
            # while E(0)'s matmuls stream, and these touch only sl_t ----
            t2s_l, tex_l = [], []
            for m in range(MCH):
                t2s = statp.tile([128, 1], f32, tag=f"t2s{m}", name=f"t2s{m}")
                nc.vector.tensor_reduce(t2s[:], sl_t[:, m, 32:48], AX.X,
                                        OP.add)
                t2s_l.append(t2s)
                tex = statp.tile([128, 1], f32, tag=f"tex{m}", name=f"tex{m}")
                nc.vector.tensor_reduce(tex[:], sl_t[:, m, 0:16], AX.X,
                                        OP.add)
                nc.vector.tensor_scalar_mul(out_t[m][:, 516:517], tex[:],
                                            1.0 / K)
                nc.vector.tensor_reduce(out_t[m][:, 517:518],
                                        sl_t[:, m, 16:32], AX.X, OP.min)

            # ---- E matmuls (fp8 DoubleRow) + per-chunk stats ----
            for m in range(MCH):
                ms, me = m * 128, (m + 1) * 128
                pe_n = []
                for n in range(NCH):
                    pe_n.append(pe_pool.tile([128, 512], f32, tag=f"pe{n}",
                                             name=f"pe{m}_{n}"))
                for j in range(JCH):
                    for n in range(NCH):
                        nc.tensor.matmul(
                            pe_n[n][:], c8_t[:, j, :, ms:me],
                            a8_t[:, j, :, n * 512:(n + 1) * 512],
                            start=(j == 0), stop=(j == JCH - 1),
                            perf_mode=DR)

                # t1 = rowdot(E, C): per-bank mul then reduce (vector engine)
                t1p = statp.tile([128, NCH], f32, tag=f"t1p{m}",
                                 name=f"t1p{m}")
                for n in range(NCH):
                    jk = junkp.tile([128, 512], bf16, tag="jk",
                                    name=f"jk{m}_{n}")
                    nc.vector.tensor_mul(jk[:], pe_n[n][:],
                                         cm_t[:, m, n * 512:(n + 1) * 512])
                    nc.vector.tensor_reduce(t1p[:, n:n + 1], jk[:], AX.X,
                                            OP.add)
                t1s = statp.tile([128, 1], f32, tag=f"t1s{m}", name=f"t1s{m}")
                nc.vector.tensor_reduce(t1s[:], t1p[:], AX.X, OP.add)

                # a_in = (t1 - t2) * invcnt
                tin = statp.tile([128, 1], f32, tag=f"tin{m}", name=f"tin{m}")
                nc.vector.tensor_sub(tin[:], t1s[:], t2s_l[m][:])
                nc.vector.tensor_mul(out_t[m][:, 513:514], tin[:],
                                     sl_t[:, m, 48:49])

                # a_out = (esum - t1) * invden
                tou = statp.tile([128, 1], f32, tag=f"to{m}", name=f"to{m}")
                nc.vector.tensor_sub(tou[:], es_t[m][:], t1s[:])
                nc.vector.tensor_mul(out_t[m][:, 514:515], tou[:],
                                     sl_t[:, m, 49:50])

                # h_glob broadcast (host-computed)
                nc.scalar.activation(out_t[m][:, D:2 * D], hg_t[:], ACT.Copy)

                nc.scalar.dma_start(out=out_d[m], in_=out_t[m][:])
    nc.compile()
    return nc


def _get_nc():
    if "nc" not in _NC_CACHE:
        _NC_CACHE["nc"] = _build_nc()
    return _NC_CACHE["nc"]


def _host_prep(h, attn_rr, attn_ro, dist_to_goal, clearance, groups):
    h = np.asarray(h, dtype=np.float32)
    attn_rr = np.asarray(attn_rr, dtype=np.float32)
    attn_ro = np.asarray(attn_ro, dtype=np.float32)
    dist_to_goal = np.asarray(dist_to_goal, dtype=np.float32)
    clearance = np.asarray(clearance, dtype=np.float32)
    groups = np.asarray(groups)

    rowsum = attn_rr.sum(axis=1)
    diag = np.ascontiguousarray(np.diagonal(attn_rr))
    hglob = h.mean(axis=0)

    # shared across cores; robot r = 256j + 128s + p -> [p, j, s, :]
    a8 = np.ascontiguousarray(
        attn_rr.astype(F8).reshape(JCH, 2, 128, N).transpose(2, 0, 1, 3))
    hr = np.concatenate(
        [h, attn_ro, (rowsum / 64.0)[:, None]], axis=1).astype(BF16)
    hr = np.ascontiguousarray(hr.reshape(KCH, 128, HRW).transpose(1, 0, 2))
    hg = np.ascontiguousarray(
        np.broadcast_to(hglob[None, :], (128, D)).astype(np.float32))

    in_maps = []
    mrow = np.arange(MLOC)[:, None]
    for s in range(NCORES):
        gs = groups[s * MLOC:(s + 1) * MLOC]
        C = np.zeros((MLOC, N), dtype=np.float32)
        np.add.at(C, (mrow, gs), 1.0)
        c8 = np.ascontiguousarray(C.T.astype(F8).reshape(
            JCH, 2, 128, MLOC).transpose(2, 0, 1, 3))
        cm = np.ascontiguousarray(
            C.astype(F8).reshape(MCH, 128, N).transpose(1, 0, 2))

        mult = C[mrow, gs]                     # (MLOC, K) slot multiplicity
        sumcc = (C * C).sum(axis=1)
        nuniq = (C > 0.0).sum(axis=1)
        sl = np.zeros((MLOC, SLW), dtype=np.float32)
        sl[:, 0:16] = dist_to_goal[gs]
        sl[:, 16:32] = clearance[gs]
        sl[:, 32:48] = diag[gs] * mult
        sl[:, 48] = 1.0 / np.maximum(K * K - sumcc, 1.0)
        sl[:, 49] = 1.0 / (K * (N - nuniq))
        sl = np.ascontiguousarray(
            sl.reshape(MCH, 128, SLW).transpose(1, 0, 2))
        in_maps.append({
            "a8": a8, "c8": c8, "cm": cm, "hr": hr, "sl": sl, "hg": hg,
        })
    return in_maps


def kernel(h, attn_rr, attn_ro, dist_to_goal, clearance, groups):
    in_maps = _host_prep(h, attn_rr, attn_ro, dist_to_goal, clearance, groups)
    nc = _get_nc()
    _NC_CACHE["last_in_maps"] = in_maps
    res = run_bass_kernel_spmd(nc, in_maps, list(range(NCORES)))
    return np.concatenate(
        [res.results[s]["out"].reshape(MLOC, FOUT) for s in range(NCORES)],
        axis=0)
